# revision 14
# baseline (speedup 1.0000x reference)
"""TRN2 Bass kernel for nn_BilinearInteraction — scheme C (no-pad, 3-engine split).

out[b,k] = sum_{e,f} E[b,i,e] W[k,e,f] E[b,j,f], 780 pairs (i<j) lexicographic,
40 fields, e=f=32.

Per core (batch shard 1024 = 8 b-tiles of 128 on partitions):
- Stage 1 (PE): per i, matmul u[b,(j,f)] = E_i^T(32e x 128b).T @ W_i(32e x Nj*32)
  in <=16-slot sub-matmuls into 2-bank PSUM super-chunks. Contraction rows sit
  at 32*(i%4) (explicit tile_position) so consecutive i's run on different
  row-groups concurrently.
- Elementwise u*Ej split across engines (GPS_FRAC of elements on GpSimd
  directly from PSUM fp32; the rest: ScalarE evicts PSUM->fp16, VectorE
  multiplies at 2x).
- Reduce f 32->1: fp16 binary tree on VectorE per half (i<11 / i>=11),
  final level adds into fp32 output row.
- Output is already in k-order: host just reshapes.
"""

import numpy as np

import concourse.bass as bass
import concourse.mybir as mybir
import concourse.tile as tile
from concourse import bacc
from concourse.bass_utils import run_bass_kernel_spmd

NF = 40
E = 32
NPAIR = 780
BATCH = 8192
NCORES = 8
B_CORE = BATCH // NCORES
NBT = B_CORE // 128            # 8 b-tiles per core

GPS_FRAC = 0.37                # fraction of elements multiplied on GpSimd
SC = 32                        # super-chunk slots (2 PSUM banks)
MM = 16                        # slots per matmul (1 bank, 512 fp32)
HALF_I = 11                    # half A: i 0..10 (374 slots), B: 11..38 (406)

NI = [NF - 1 - i for i in range(NF - 1)]            # pairs per i
SBASE = np.concatenate([[0], np.cumsum(NI)]).astype(int)   # slot base per i
assert SBASE[NF - 1] == NPAIR
SA = int(SBASE[HALF_I])        # 374
SB = NPAIR - SA                # 406

# per-residue W packing offsets
POSR = {}
_rlen = [0, 0, 0, 0]
for _i in range(NF - 1):
    _r = _i % 4
    POSR[_i] = _rlen[_r]
    _rlen[_r] += NI[_i]
WL = max(_rlen)                # 210


def _pack_w(W):
    wp = np.zeros((128, WL, E), np.float32)
    for i in range(NF - 1):
        r = i % 4
        n = NI[i]
        k0 = SBASE[i]
        wp[32 * r:32 * r + 32, POSR[i]:POSR[i] + n, :] = (
            W[k0:k0 + n].transpose(1, 0, 2))
    return wp


def _pack_et(emb):
    # [NC, B, 10, 4, E] -> [NC, 4(r), E(e), 10(m), B]
    et = emb.reshape(NCORES, B_CORE, NF // 4, 4, E).transpose(0, 3, 4, 2, 1)
    return np.ascontiguousarray(et.reshape(NCORES, 128, NF // 4, B_CORE))


# ---------------- bass program ----------------
_CACHED = None


def _build():
    global _CACHED
    if _CACHED is not None:
        return _CACHED

    nc = bacc.Bacc("TRN2", target_bir_lowering=False, debug=False)
    f32 = mybir.dt.float32
    f16 = mybir.dt.float16

    et16_d = nc.dram_tensor("et16", [128, NF // 4, B_CORE], f16, kind="ExternalInput")
    wp_d = nc.dram_tensor("wp", [128, WL, E], f16, kind="ExternalInput")
    e16n_d = nc.dram_tensor("e16n", [NBT, 128, NF, E], f16, kind="ExternalInput")
    o_d = nc.dram_tensor("o", [NBT, 128, NPAIR], f32, kind="ExternalOutput")

    # build the static super-chunk schedule per half: (i, off, nj, engine)
    # engine: 0 = scalar-evict + vector-mul, 1 = scalar-evict + gpsimd-mul.
    # GpSimd's share is front-loaded in each half so its slow tail doesn't
    # delay the reduction-tree barrier.
    sched = {0: [], 1: []}
    for half, (ilo, ihi) in enumerate([(0, HALF_I), (HALF_I, NF - 1)]):
        half_elems = sum(NI[i] for i in range(ilo, ihi)) * E
        gps = 0
        for i in range(ilo, ihi):
            for off in range(0, NI[i], SC):
                nj = min(SC, NI[i] - off)
                ne = nj * E
                if gps + ne <= GPS_FRAC * half_elems:
                    eng = 1
                    gps += ne
                else:
                    eng = 0
                sched[half].append((i, off, nj, eng))

    with tile.TileContext(nc) as tc:
        with (
            tc.tile_pool(name="consts", bufs=1) as consts,
            tc.tile_pool(name="en", bufs=2) as en,
            tc.tile_pool(name="ued", bufs=6) as uedp,
            tc.tile_pool(name="ueg", bufs=4) as uegp,
            tc.tile_pool(name="vpool", bufs=1) as vpool,
            tc.tile_pool(name="tree", bufs=2) as tree,
            tc.tile_pool(name="outs", bufs=2) as outs,
            tc.tile_pool(name="upsum", bufs=4, space="PSUM") as upsum,
        ):
            wp_sb = consts.tile([128, WL, E], f16)
            for s in range(0, WL, 53):
                e2 = min(s + 53, WL)
                nc.sync.dma_start(out=wp_sb[:, s:e2, :], in_=wp_d[:, s:e2, :])
            et16_sb = consts.tile([128, NF // 4, B_CORE], f16)
            for m in range(NF // 4):
                nc.sync.dma_start(out=et16_sb[:, m, :], in_=et16_d[:, m, :])

            for bt in range(NBT):
                bs = bass.ts(bt, 128)
                e16n = en.tile([128, NF, E], f16, tag="e16n")
                nc.sync.dma_start(out=e16n[:], in_=e16n_d[bt, :, :, :])
                obt = outs.tile([128, NPAIR], f32, tag="obt")

                for half in range(2):
                    if half == 0:
                        ilo, ihi, hbase, Sh = 0, HALF_I, 0, SA
                    else:
                        ilo, ihi, hbase, Sh = HALF_I, NF - 1, SA, SB
                    vh = vpool.tile([128, SB, E], f16, tag=f"v{half}")

                    for (i, off, nj, eng) in sched[half]:
                        r = i % 4
                        j0 = i + 1 + off
                        sb0 = int(SBASE[i]) + off - hbase
                        u_ps = upsum.tile([128, SC, E], f32, tag="u")
                        for c0 in range(0, nj, MM):
                            n1 = min(MM, nj - c0)
                            nc.tensor.matmul(
                                u_ps[:, c0:c0 + n1, :],
                                et16_sb[32 * r:32 * r + 32, i // 4, bs],
                                wp_sb[32 * r:32 * r + 32,
                                      POSR[i] + off + c0:POSR[i] + off + c0 + n1, :],
                                start=True,
                                stop=True,
                                tile_position=(32 * r, 0),
                            )
                        if eng == 1:
                            ue = uegp.tile([128, SC, E], f16, tag="ueg")
                        else:
                            ue = uedp.tile([128, SC, E], f16, tag="ued")
                        nc.scalar.copy(out=ue[:, :nj, :], in_=u_ps[:, :nj, :])
                        meng = nc.gpsimd if eng == 1 else nc.vector
                        meng.tensor_mul(
                            vh[:, sb0:sb0 + nj, :],
                            ue[:, :nj, :],
                            e16n[:, j0:j0 + nj, :],
                        )

                    s1 = tree.tile([128, SB, 16], f16, tag="s1")
                    nc.vector.tensor_add(
                        s1[:, :Sh, :], vh[:, :Sh, 0:16], vh[:, :Sh, 16:32])
                    s2 = tree.tile([128, SB, 8], f16, tag="s2")
                    nc.vector.tensor_add(
                        s2[:, :Sh, :], s1[:, :Sh, 0:8], s1[:, :Sh, 8:16])
                    s3 = tree.tile([128, SB, 4], f16, tag="s3")
                    nc.vector.tensor_add(
                        s3[:, :Sh, :], s2[:, :Sh, 0:4], s2[:, :Sh, 4:8])
                    s4 = tree.tile([128, SB, 2], f16, tag="s4")
                    nc.vector.tensor_add(
                        s4[:, :Sh, :], s3[:, :Sh, 0:2], s3[:, :Sh, 2:4])
                    nc.vector.tensor_add(
                        obt[:, hbase:hbase + Sh],
                        s4[:, :Sh, 0],
                        s4[:, :Sh, 1],
                    )

                nc.sync.dma_start(out=o_d[bt, :, :], in_=obt[:])

    nc.compile()
    _CACHED = nc
    return nc


# ---------------- public entry ----------------
def _run(embeddings, W, **spmd_kwargs):
    embeddings = np.ascontiguousarray(np.asarray(embeddings, dtype=np.float32))
    W = np.ascontiguousarray(np.asarray(W, dtype=np.float32))

    et16 = _pack_et(embeddings).astype(np.float16)
    e16n = np.ascontiguousarray(
        embeddings.reshape(NCORES, NBT, 128, NF, E).astype(np.float16))
    wp = _pack_w(W).astype(np.float16)

    nc = _build()
    in_maps = [
        {"et16": et16[c], "wp": wp, "e16n": e16n[c]}
        for c in range(NCORES)
    ]
    res = run_bass_kernel_spmd(nc, in_maps, list(range(NCORES)), **spmd_kwargs)

    out = np.empty((BATCH, NPAIR), np.float32)
    for c in range(NCORES):
        out[c * B_CORE:(c + 1) * B_CORE] = res.results[c]["o"].reshape(B_CORE, NPAIR)
    return out, res


def kernel(embeddings, W):
    out, _ = _run(embeddings, W)
    return out


# revision 20
# speedup vs baseline: 1.0220x; 1.0220x over previous
"""TRN2 Bass kernel for nn_BilinearInteraction — scheme D (range-local trees).

out[b,k] = sum_{e,f} E[b,i,e] W[k,e,f] E[b,j,f], 780 pairs (i<j) lexicographic,
40 fields, e=f=32.

Per core (batch shard 1024 = 8 b-tiles of 128 on partitions):
- Stage 1 (PE): per i, u[b,(j,f)] = E_i^T(32e x 128b).T @ W_i(32e x Nj*32)
  in per-i chunks of <=32 slots (PSUM tile holds ONE i only: concurrent
  row-group matmuls must not share a PSUM bank). Contraction rows at
  32*(i%4) via explicit tile_position -> 4-way PE concurrency.
- Elementwise u*Ej split across engines per chunk:
    eng 0: ScalarE evicts PSUM->fp16, VectorE multiplies (2x mode)
    eng 1: ScalarE evicts PSUM->fp16, GpSimd multiplies
    eng 2: VectorE multiplies directly from PSUM fp32 (1x, no evict)
- Reduce f 32->1: fp16 binary tree on VectorE per RANGE of ~128 slots,
  emitted right after the range's multiplies (short dependency horizon,
  no half-wide barriers). Final level adds into fp32 output row.
- Output is already in k-order: host just reshapes.
"""

import numpy as np

import concourse.bass as bass
import concourse.mybir as mybir
import concourse.tile as tile
from concourse import bacc
from concourse.bass_utils import run_bass_kernel_spmd

NF = 40
E = 32
NPAIR = 780
BATCH = 8192
NCORES = 8
B_CORE = BATCH // NCORES
NBT = B_CORE // 128            # 8 b-tiles per core

GPS_FRAC = 0.32                # fraction of elements multiplied on GpSimd
DIR_FRAC = 0.10                # fraction multiplied by Vector direct from PSUM
SC = 32                        # chunk slots (<=2 PSUM banks, single i)
MM = 16                        # slots per matmul (1 bank, 512 fp32)
RNG = 128                      # slots per reduction range

NI = [NF - 1 - i for i in range(NF - 1)]            # pairs per i
SBASE = np.concatenate([[0], np.cumsum(NI)]).astype(int)   # slot base per i
assert SBASE[NF - 1] == NPAIR

# per-residue W packing offsets
POSR = {}
_rlen = [0, 0, 0, 0]
for _i in range(NF - 1):
    _r = _i % 4
    POSR[_i] = _rlen[_r]
    _rlen[_r] += NI[_i]
WL = max(_rlen)                # 210


def _pack_w(W):
    wp = np.zeros((128, WL, E), np.float32)
    for i in range(NF - 1):
        r = i % 4
        n = NI[i]
        k0 = SBASE[i]
        wp[32 * r:32 * r + 32, POSR[i]:POSR[i] + n, :] = (
            W[k0:k0 + n].transpose(1, 0, 2))
    return wp


def _pack_et(emb):
    # [NC, B, 10, 4, E] -> [NC, 4(r), E(e), 10(m), B]
    et = emb.reshape(NCORES, B_CORE, NF // 4, 4, E).transpose(0, 3, 4, 2, 1)
    return np.ascontiguousarray(et.reshape(NCORES, 128, NF // 4, B_CORE))


# ---------------- static schedule ----------------
# chunks: (i, ioff, n, slot0) with n <= SC, single i each.
_chunks = []
for _i in range(NF - 1):
    for _off in range(0, NI[_i], SC):
        _n = min(SC, NI[_i] - _off)
        _chunks.append((_i, _off, _n, int(SBASE[_i]) + _off))

# ranges: groups of consecutive chunks covering ~RNG slots; tree is emitted
# per range right after its multiplies.
RANGES = []                    # (slot0, nslots, [chunk indices])
_cur = []
_cslots = 0
for _ci, (_i, _off, _n, _s0) in enumerate(_chunks):
    _cur.append(_ci)
    _cslots += _n
    if _cslots >= RNG or _ci == len(_chunks) - 1:
        RANGES.append((_chunks[_cur[0]][3], _cslots, list(_cur)))
        _cur, _cslots = [], 0
RMAX = max(r[1] for r in RANGES)

# engine per chunk: greedy running-fraction; GpSimd never takes the last
# chunk of a range (its slow tail would gate the range's tree).
ENG = {}
_gps = _dir = _tot = 0
for (_s0, _ns, _cis) in RANGES:
    for _k, _ci in enumerate(_cis):
        _ne = _chunks[_ci][2] * E
        last = (_k == len(_cis) - 1)
        if not last and _gps + _ne <= GPS_FRAC * (_tot + _ne):
            ENG[_ci] = 1
            _gps += _ne
        elif _dir + _ne <= DIR_FRAC * (_tot + _ne):
            ENG[_ci] = 2
            _dir += _ne
        else:
            ENG[_ci] = 0
        _tot += _ne


# ---------------- bass program ----------------
_CACHED = None


def _build():
    global _CACHED
    if _CACHED is not None:
        return _CACHED

    nc = bacc.Bacc("TRN2", target_bir_lowering=False, debug=False)
    f32 = mybir.dt.float32
    f16 = mybir.dt.float16

    et16_d = nc.dram_tensor("et16", [128, NF // 4, B_CORE], f16, kind="ExternalInput")
    wp_d = nc.dram_tensor("wp", [128, WL, E], f16, kind="ExternalInput")
    e16n_d = nc.dram_tensor("e16n", [NBT, 128, NF, E], f16, kind="ExternalInput")
    o_d = nc.dram_tensor("o", [NBT, 128, NPAIR], f32, kind="ExternalOutput")

    with tile.TileContext(nc) as tc:
        with (
            tc.tile_pool(name="consts", bufs=1) as consts,
            tc.tile_pool(name="en", bufs=2) as en,
            tc.tile_pool(name="ued", bufs=6) as uedp,
            tc.tile_pool(name="ueg", bufs=4) as uegp,
            tc.tile_pool(name="vpool", bufs=3) as vpool,
            tc.tile_pool(name="tree", bufs=3) as tree,
            tc.tile_pool(name="outs", bufs=2) as outs,
            tc.tile_pool(name="upsum", bufs=3, space="PSUM") as upsum,
            tc.tile_pool(name="dpsum", bufs=1, space="PSUM") as dpsum,
        ):
            wp_sb = consts.tile([128, WL, E], f16)
            for s in range(0, WL, 53):
                e2 = min(s + 53, WL)
                nc.sync.dma_start(out=wp_sb[:, s:e2, :], in_=wp_d[:, s:e2, :])
            et16_sb = consts.tile([128, NF // 4, B_CORE], f16)
            for m in range(NF // 4):
                nc.sync.dma_start(out=et16_sb[:, m, :], in_=et16_d[:, m, :])

            for bt in range(NBT):
                bs = bass.ts(bt, 128)
                e16n = en.tile([128, NF, E], f16, tag="e16n")
                nc.sync.dma_start(out=e16n[:], in_=e16n_d[bt, :, :, :])
                obt = outs.tile([128, NPAIR], f32, tag="obt")

                for (rs0, rns, cis) in RANGES:
                    vr = vpool.tile([128, RMAX, E], f16, tag="vr")
                    for ci in cis:
                        (i, ioff, n, s0) = _chunks[ci]
                        eng = ENG[ci]
                        r = i % 4
                        pool_ = dpsum if eng == 2 else upsum
                        u_ps = pool_.tile([128, SC, E], f32,
                                          tag="ud" if eng == 2 else "u")
                        for c0 in range(0, n, MM):
                            n1 = min(MM, n - c0)
                            nc.tensor.matmul(
                                u_ps[:, c0:c0 + n1, :],
                                et16_sb[32 * r:32 * r + 32, i // 4, bs],
                                wp_sb[32 * r:32 * r + 32,
                                      POSR[i] + ioff + c0:
                                      POSR[i] + ioff + c0 + n1, :],
                                start=True,
                                stop=True,
                                tile_position=(32 * r, 0),
                            )
                        if eng == 2:
                            src = u_ps
                        else:
                            if eng == 1:
                                src = uegp.tile([128, SC, E], f16, tag="ueg")
                            else:
                                src = uedp.tile([128, SC, E], f16, tag="ued")
                            nc.scalar.copy(out=src[:, :n, :], in_=u_ps[:, :n, :])
                        meng = nc.gpsimd if eng == 1 else nc.vector
                        meng.tensor_mul(
                            vr[:, s0 - rs0:s0 - rs0 + n, :],
                            src[:, :n, :],
                            e16n[:, i + 1 + ioff:i + 1 + ioff + n, :],
                        )

                    # range-local fp16 reduction tree on VectorE
                    S = rns
                    s1 = tree.tile([128, RMAX, 16], f16, tag="s1")
                    nc.vector.tensor_add(
                        s1[:, :S, :], vr[:, :S, 0:16], vr[:, :S, 16:32])
                    s2 = tree.tile([128, RMAX, 8], f16, tag="s2")
                    nc.vector.tensor_add(
                        s2[:, :S, :], s1[:, :S, 0:8], s1[:, :S, 8:16])
                    s3 = tree.tile([128, RMAX, 4], f16, tag="s3")
                    nc.vector.tensor_add(
                        s3[:, :S, :], s2[:, :S, 0:4], s2[:, :S, 4:8])
                    s4 = tree.tile([128, RMAX, 2], f16, tag="s4")
                    nc.vector.tensor_add(
                        s4[:, :S, :], s3[:, :S, 0:2], s3[:, :S, 2:4])
                    nc.vector.tensor_add(
                        obt[:, rs0:rs0 + S],
                        s4[:, :S, 0],
                        s4[:, :S, 1],
                    )

                nc.sync.dma_start(out=o_d[bt, :, :], in_=obt[:])

    nc.compile()
    _CACHED = nc
    return nc


# ---------------- public entry ----------------
def _run(embeddings, W, **spmd_kwargs):
    embeddings = np.ascontiguousarray(np.asarray(embeddings, dtype=np.float32))
    W = np.ascontiguousarray(np.asarray(W, dtype=np.float32))

    et16 = _pack_et(embeddings).astype(np.float16)
    e16n = np.ascontiguousarray(
        embeddings.reshape(NCORES, NBT, 128, NF, E).astype(np.float16))
    wp = _pack_w(W).astype(np.float16)

    nc = _build()
    in_maps = [
        {"et16": et16[c], "wp": wp, "e16n": e16n[c]}
        for c in range(NCORES)
    ]
    res = run_bass_kernel_spmd(nc, in_maps, list(range(NCORES)), **spmd_kwargs)

    out = np.empty((BATCH, NPAIR), np.float32)
    for c in range(NCORES):
        out[c * B_CORE:(c + 1) * B_CORE] = res.results[c]["o"].reshape(B_CORE, NPAIR)
    return out, res


def kernel(embeddings, W):
    out, _ = _run(embeddings, W)
    return out
